# revision 1
# baseline (speedup 1.0000x reference)
"""DeeperGCN forward on 8 TRN2 NeuronCores (Bass/Tile).

Sharding: dst-partitioned graph parallel. Core k owns original nodes
[12500k, 12500(k+1)), padded to 12544 = 98 blocks of 128. Edges live on the
core owning their dst, laid densely into 128-edge chunks per dst-block. Per
layer:
  x = relu(LN(h)) (batched big-tile ops, bf16) -> AllGather x -> batched
  indirect gather of x[src] (one DMA per ~32-chunk group) -> messages
  m = relu(x[src]+eh), e = exp(beta*m) -> per-dst per-feature softmax sums
  via one-hot matmuls accumulated in PSUM (PE does the segment reduction;
  [e|m*e] packed in one 128-col rhs) -> y = num/den + x -> GENConv MLP per
  block -> h += y.
Node/edge encoders (h0, eh) and the one-hot matrices are computed host-side
and uploaded. Pad slots carry eh = -1e9 (message exactly 0) and a zero
one-hot column, so they contribute nothing.
Softmax uses no segment-max: logits are bounded (LN output <= sqrt(63)), so
exp() cannot overflow, and e/s is shift-invariant so results match the
reference to fp rounding.
"""
import json
import numpy as np
import ml_dtypes

import concourse.bass as bass
import concourse.mybir as mybir
import concourse.tile as tile
from concourse.bass_types import AP
from concourse.bass_utils import run_bass_kernel_spmd
from concourse.masks import make_identity

# ---- problem constants (hardcoded per contract) ----
N_NODES = 100000
N_EDGES = 1200000
NODE_DIM = 128
EDGE_DIM = 8
HID = 64
OUT_DIM = 112
L_LAYERS = 7
EPS_MSG = 1e-7
LN_EPS = 1e-5

N_CORES = 8
N_PER_CORE = 12500          # original nodes per core
NBLK = 98                   # dst blocks per core (128 dsts each)
NLOC = NBLK * 128           # 12544 padded local nodes
NFULL = N_CORES * NLOC      # padded global rows in gathered x
JMAX = 24                   # max chunks per gather group
F32 = mybir.dt.float32
BF16 = mybir.dt.bfloat16
I32 = mybir.dt.int32
BIGNEG = -1e9


def _split_multi_waits(bir_bytes: bytes) -> bytes:
    """Walrus in this container allows only ONE semaphore wait per
    instruction: hoist extra waits onto same-engine NoOps."""
    d = json.loads(bir_bytes)
    ctr = 0
    for f in d["functions"]:
        for blk in f["blocks"]:
            insts = blk["instructions"]
            out = []
            changed = False
            for inst in insts:
                si = inst.get("sync_info")
                if si:
                    waits = si.get("on_wait") or []
                    if len(waits) > 1:
                        changed = True
                        for w in waits[:-1]:
                            ctr += 1
                            out.append({
                                "debug": inst.get("debug", 0),
                                "engine": inst["engine"],
                                "ins": [], "outs": [],
                                "name": f"I-wsplit-{ctr}",
                                "opcode": "NoOp",
                                "sync_info": {"on_wait": [w], "on_update": []},
                            })
                        si["on_wait"] = waits[-1:]
                out.append(inst)
            if changed:
                blk["instructions"] = out
    return json.dumps(d).encode()


def _install_wait_split(nc):
    orig = nc.to_json_bytes
    nc.to_json_bytes = lambda: _split_multi_waits(orig())


def _bcast_mid(ap, n):
    """[128, D] AP -> [128, n(bcast), D] with 0-stride middle axis."""
    return AP(ap.tensor, ap.offset, [list(ap.ap[0]), [0, n], list(ap.ap[1])])


# ---------------------------------------------------------------- host prep
def host_prep(node_feats, edge_feats, src, dst, W_node, b_node, W_edge, b_edge):
    """Per-core dense edge-chunk layout + host-side encoders + one-hots."""
    h0v = node_feats.astype(np.float32) @ W_node + b_node          # [N, HID]
    ehv = edge_feats.astype(np.float32) @ W_edge + b_edge          # [E, HID]

    owner = dst // N_PER_CORE
    cores = []
    for k in range(N_CORES):
        sel = np.nonzero(owner == k)[0]
        dl = dst[sel] - k * N_PER_CORE
        blk = dl // 128
        eidx = np.argsort(blk, kind="stable")
        cores.append(dict(sel=sel[eidx], dl=dl[eidx], blks=blk[eidx],
                          counts=np.bincount(blk, minlength=NBLK)))

    caps = np.max(np.stack([(c["counts"] + 127) // 128 for c in cores]),
                  axis=0)                               # [NBLK] chunks/block
    caps = np.maximum(caps, 1)
    # group consecutive blocks: sum of caps <= JMAX per group
    groups = []                                         # (b0, nb, [caps...])
    b0 = 0
    while b0 < NBLK:
        nb, tot = 0, 0
        while b0 + nb < NBLK and (nb == 0 or tot + caps[b0 + nb] <= JMAX):
            tot += caps[b0 + nb]
            nb += 1
        groups.append((b0, nb, [int(caps[b0 + i]) for i in range(nb)]))
        b0 += nb
    start_chunk = np.zeros(NBLK + 1, np.int64)
    start_chunk[1:] = np.cumsum(caps)
    ncht = int(start_chunk[-1])
    ncht_pad = ncht + JMAX

    for k, c in enumerate(cores):
        sel, dl, blks = c["sel"], c["dl"], c["blks"]
        starts = np.zeros(NBLK, np.int64)
        starts[1:] = np.cumsum(c["counts"])[:-1]
        ei = np.arange(len(blks)) - starts[blks]        # index within block
        cinb = ei // 128
        p = ei % 128
        chunk = start_chunk[blks] + cinb
        drow = dl % 128

        est = np.zeros((128, ncht, HID + 128), ml_dtypes.bfloat16)
        est[:, :, 0:HID] = ml_dtypes.bfloat16(BIGNEG)
        est[p, chunk, 0:HID] = ehv[sel].astype(ml_dtypes.bfloat16)
        est[p, chunk, HID + drow] = 1.0
        s = src[sel]
        sidx = np.zeros((128, ncht_pad), np.int32)
        sidx[p, chunk] = ((s // N_PER_CORE) * NLOC +
                          (s % N_PER_CORE)).astype(np.int32)
        h0 = np.zeros((NLOC, HID), np.float32)
        h0[:N_PER_CORE] = h0v[k * N_PER_CORE:(k + 1) * N_PER_CORE]
        c.clear()
        c.update(est=est, sidx=sidx,
                 h0=np.ascontiguousarray(
                     h0.reshape(NBLK, 128, HID).transpose(1, 0, 2)))
    return cores, groups, ncht


# ---------------------------------------------------------------- device build
def build_bass(weights, groups, ncht, n_layers=L_LAYERS):
    (betas, W1, b1, ln1_g, ln1_b, W2, b2, norm_g, norm_b, W_out, b_out) = weights
    nc = bass.Bass("TRN2", target_bir_lowering=False, debug=False,
                   num_devices=N_CORES)

    h0_d = nc.dram_tensor("h0", [128, NBLK * HID], F32, kind="ExternalInput")
    est_d = nc.dram_tensor("est", [128, ncht * (HID + 128)], BF16,
                           kind="ExternalInput")
    sidx_d = nc.dram_tensor("sidx", [128, ncht + JMAX], I32,
                            kind="ExternalInput")
    wb_d = {}
    for name, arr in [
        ("w1", W1.transpose(1, 0, 2)),                 # [HID, L, 2H]
        ("w2", W2.transpose(1, 0, 2)),                 # [2H, L, HID]
        ("wout", W_out),
        ("b1", np.broadcast_to(b1[:, None, :], (L_LAYERS, 128, 2 * HID)).transpose(1, 0, 2)),
        ("l1g", np.broadcast_to(ln1_g[:, None, :], (L_LAYERS, 128, 2 * HID)).transpose(1, 0, 2)),
        ("l1b", np.broadcast_to(ln1_b[:, None, :], (L_LAYERS, 128, 2 * HID)).transpose(1, 0, 2)),
        ("b2", np.broadcast_to(b2[:, None, :], (L_LAYERS, 128, HID)).transpose(1, 0, 2)),
        ("ng", np.broadcast_to(norm_g[:, None, :], (L_LAYERS, 128, HID)).transpose(1, 0, 2)),
        ("nb", np.broadcast_to(norm_b[:, None, :], (L_LAYERS, 128, HID)).transpose(1, 0, 2)),
        ("bout", np.broadcast_to(b_out, (128, OUT_DIM))),
    ]:
        a = np.ascontiguousarray(arr, np.float32)
        wb_d[name] = nc.dram_tensor(name, list(a.shape), F32, kind="ExternalInput")
        wb_d[f"_{name}_np"] = a
    out_d = nc.dram_tensor("out", [NLOC, OUT_DIM], F32, kind="ExternalOutput")

    jgmax = max(sum(cs) for _, _, cs in groups)

    with tile.TileContext(nc) as tc:
        with tc.tile_pool(name="persist", bufs=1) as pp, \
             tc.tile_pool(name="dram", bufs=1, space="DRAM") as dram, \
             tc.tile_pool(name="work", bufs=2) as wk, \
             tc.tile_pool(name="nodeops", bufs=3) as nop, \
             tc.tile_pool(name="pseg", bufs=4, space="PSUM") as pseg, \
             tc.tile_pool(name="pmlp", bufs=1, space="PSUM") as pmlp:

            # ---- persistent SBUF state ----
            h_sb = pp.tile([128, NBLK, HID], F32)
            nc.sync.dma_start(out=h_sb[:].rearrange("p b k -> p (b k)"),
                              in_=h0_d[:, :])
            xbf = pp.tile([128, NBLK, HID], BF16)
            # sn: [p, b, 0:HID]=den (also LN scratch sq);
            #     [p, b, HID:2H]=num -> y (also LN scratch u)
            sn = pp.tile([128, NBLK, 2 * HID], F32)
            sidx_sb = pp.tile([128, ncht + JMAX], I32)
            nc.sync.dma_start(out=sidx_sb[:], in_=sidx_d[:, :])
            ident = pp.tile([128, 128], F32)
            make_identity(nc, ident[:])
            epsq_sb = pp.tile([128, 1], F32, name="epsq")
            nc.vector.memset(epsq_sb[:], float(HID) * HID * LN_EPS)
            eps_sb = pp.tile([128, 1], F32, name="eps1")
            nc.vector.memset(eps_sb[:], LN_EPS)

            w1_sb = pp.tile([HID, L_LAYERS, 2 * HID], F32)
            nc.sync.dma_start(out=w1_sb[:], in_=wb_d["w1"][:, :, :])
            w2_sb = pp.tile([2 * HID, L_LAYERS, HID], F32)
            nc.sync.dma_start(out=w2_sb[:], in_=wb_d["w2"][:, :, :])
            wout_sb = pp.tile([HID, OUT_DIM], F32)
            nc.sync.dma_start(out=wout_sb[:], in_=wb_d["wout"][:, :])
            bias_sb = {}
            for nm, dd in [("b1", 2 * HID), ("l1g", 2 * HID), ("l1b", 2 * HID),
                           ("b2", HID), ("ng", HID), ("nb", HID)]:
                bias_sb[nm] = pp.tile([128, L_LAYERS, dd], F32, name=f"bs_{nm}")
                nc.sync.dma_start(out=bias_sb[nm][:], in_=wb_d[nm][:, :, :])
            bias_sb["bout"] = pp.tile([128, OUT_DIM], F32, name="bs_bout")
            nc.sync.dma_start(out=bias_sb["bout"][:], in_=wb_d["bout"][:, :])

            # ---- DRAM internals: collective buffers, one pair per layer ----
            xins = [dram.tile([NLOC, HID], F32, name=f"xin{i}")
                    for i in range(n_layers)]
            xfulls = [dram.tile([NFULL, HID], F32, addr_space="Shared",
                                name=f"xfull{i}") for i in range(n_layers)]

            D = float(HID)
            ADD = mybir.AluOpType.add
            MUL = mybir.AluOpType.mult
            SUB = mybir.AluOpType.subtract

            def _sn_half(off, b0=0, nb=NBLK):
                full = sn[:]
                return AP(full.tensor, full.offset + off + b0 * 2 * HID,
                          [list(full.ap[0]), [2 * HID, nb], [1, HID]])

            sq_ap = _sn_half(0)          # den half as LN scratch
            u_ap = _sn_half(HID)         # num half as LN scratch

            def batched_ln_relu(src_tile, g_ap, b_ap, out_ap):
                """out = relu(LN(src) * g + b), batched over all NBLK blocks.
                src [128, NBLK, HID] f32; g/b [128, HID] (per-feature).
                Uses sn as scratch (sq then u)."""
                s1 = nop.tile([128, NBLK], F32, name="ln_s1")
                nc.vector.tensor_reduce(out=s1[:], in_=src_tile[:],
                                        axis=mybir.AxisListType.X, op=ADD)
                nc.vector.tensor_tensor(out=sq_ap, in0=src_tile[:],
                                        in1=src_tile[:], op=MUL)
                s2 = nop.tile([128, NBLK], F32, name="ln_s2")
                nc.vector.tensor_reduce(out=s2[:], in_=sq_ap,
                                        axis=mybir.AxisListType.X, op=ADD)
                t1 = nop.tile([128, NBLK], F32, name="ln_t1")
                nc.vector.tensor_tensor(out=t1[:], in0=s1[:], in1=s1[:],
                                        op=MUL)
                # q = D*S2 - S1^2  (= D^2 * var)
                q = nop.tile([128, NBLK], F32, name="ln_q")
                nc.vector.scalar_tensor_tensor(out=q[:], in0=s2[:], scalar=D,
                                               in1=t1[:], op0=MUL, op1=SUB)
                # r = 1/sqrt(q + D^2*eps);  A = D*r;  B = S1*r
                qs = nop.tile([128, NBLK], F32, name="ln_qs")
                nc.scalar.activation(out=qs[:], in_=q[:],
                                     func=mybir.ActivationFunctionType.Sqrt,
                                     bias=epsq_sb[:])
                r = nop.tile([128, NBLK], F32, name="ln_r")
                nc.vector.reciprocal(out=r[:], in_=qs[:])
                a = nop.tile([128, NBLK], F32, name="ln_a")
                nc.vector.tensor_scalar_mul(out=a[:], in0=r[:], scalar1=D)
                bb = nop.tile([128, NBLK], F32, name="ln_b")
                nc.vector.tensor_tensor(out=bb[:], in0=s1[:], in1=r[:],
                                        op=MUL)
                # u = h*A - B ; v = u*g + b ; out = relu(v)
                nc.vector.tensor_tensor(out=u_ap, in0=src_tile[:],
                                        in1=a[:].broadcast_to([128, NBLK, HID]),
                                        op=MUL)
                nc.vector.tensor_tensor(out=u_ap, in0=u_ap,
                                        in1=bb[:].broadcast_to([128, NBLK, HID]),
                                        op=SUB)
                nc.vector.tensor_tensor(out=u_ap, in0=u_ap,
                                        in1=_bcast_mid(g_ap, NBLK), op=MUL)
                nc.vector.tensor_tensor(out=u_ap, in0=u_ap,
                                        in1=_bcast_mid(b_ap, NBLK), op=ADD)
                nc.scalar.activation(out=out_ap, in_=u_ap,
                                     func=mybir.ActivationFunctionType.Relu)

            # ================= layers =================
            for li in range(n_layers):
                beta = float(betas[li])
                xin, xfull = xins[li], xfulls[li]

                # ---- x = relu(LN(h)) ----
                batched_ln_relu(h_sb, bias_sb["ng"][:, li, :],
                                bias_sb["nb"][:, li, :], u_ap)
                nc.sync.dma_start(
                    out=xin[:, :].rearrange("(b p) k -> p b k", p=128),
                    in_=u_ap)
                nc.vector.tensor_copy(out=xbf[:], in_=u_ap)
                # ---- all-gather x ----
                nc.gpsimd.collective_compute(
                    "AllGather", mybir.AluOpType.bypass,
                    replica_groups=[list(range(N_CORES))],
                    ins=[xin[:, :].opt()], outs=[xfull[:, :].opt()])

                # ---- MLP per block: h += W2 @ relu(LN1(y@W1+b1)) + b2 ----
                def mlp_block(b):
                    pyt = pmlp.tile([HID, 128], F32, name="pyt", tag="ptr",
                                    bufs=2)
                    nc.tensor.transpose(out=pyt[:], in_=sn[:, b, HID:2 * HID],
                                        identity=ident[:])
                    yt = nop.tile([HID, 128], F32, name="yt")
                    nc.scalar.activation(out=yt[:], in_=pyt[:],
                                         func=mybir.ActivationFunctionType.Copy)
                    pm1 = pmlp.tile([128, 2 * HID], F32, name="pm1", tag="pmm",
                                    bufs=2)
                    nc.tensor.matmul(out=pm1[:], lhsT=yt[:],
                                     rhs=w1_sb[:, li, :], start=True, stop=True)
                    z0 = nop.tile([128, 2 * HID], F32, name="z0")
                    nc.vector.tensor_tensor(out=z0[:], in0=pm1[:],
                                            in1=bias_sb["b1"][:, li, :],
                                            op=ADD)
                    st1 = nop.tile([128, 6], F32, name="st1")
                    nc.vector.bn_stats(out=st1[:], in_=z0[:])
                    mv1 = nop.tile([128, 2], F32, name="mv1")
                    nc.vector.bn_aggr(out=mv1[:], in_=st1[:])
                    sd = nop.tile([128, 1], F32, name="sd")
                    nc.scalar.activation(out=sd[:], in_=mv1[:, 1:2],
                                         func=mybir.ActivationFunctionType.Sqrt,
                                         bias=eps_sb[:])
                    rstd = nop.tile([128, 1], F32, name="rstd")
                    nc.vector.reciprocal(out=rstd[:], in_=sd[:])
                    z1 = nop.tile([128, 2 * HID], F32, name="z1")
                    nc.vector.scalar_tensor_tensor(
                        out=z1[:], in0=z0[:], scalar=mv1[:, 0:1],
                        in1=rstd[:].broadcast_to([128, 2 * HID]),
                        op0=SUB, op1=MUL)
                    nc.gpsimd.tensor_tensor(out=z1[:], in0=z1[:],
                                            in1=bias_sb["l1g"][:, li, :],
                                            op=MUL)
                    nc.gpsimd.tensor_tensor(out=z1[:], in0=z1[:],
                                            in1=bias_sb["l1b"][:, li, :],
                                            op=ADD)
                    nc.scalar.activation(out=z1[:], in_=z1[:],
                                         func=mybir.ActivationFunctionType.Relu)
                    pzt = pmlp.tile([128, 128], F32, name="pzt", tag="ptr",
                                    bufs=2)
                    nc.tensor.transpose(out=pzt[:], in_=z1[:], identity=ident[:])
                    zt = nop.tile([128, 128], F32, name="zt")
                    nc.scalar.activation(out=zt[:], in_=pzt[:],
                                         func=mybir.ActivationFunctionType.Copy)
                    pm2 = pmlp.tile([128, HID], F32, name="pm2", tag="pmm",
                                    bufs=2)
                    nc.tensor.matmul(out=pm2[:], lhsT=zt[:],
                                     rhs=w2_sb[:, li, :], start=True, stop=True)
                    y2 = nop.tile([128, HID], F32, name="y2")
                    nc.vector.tensor_tensor(out=y2[:], in0=pm2[:],
                                            in1=bias_sb["b2"][:, li, :],
                                            op=ADD)
                    nc.gpsimd.tensor_tensor(out=h_sb[:, b, :],
                                            in0=h_sb[:, b, :], in1=y2[:],
                                            op=ADD)


                # ---- edge phase: per group ----
                c0 = 0
                EW = HID + 128
                for gi, (b0, nb, cs) in enumerate(groups):
                    jg = sum(cs)
                    est = wk.tile([128, jgmax, EW], BF16, name="est", bufs=3)
                    eng = nc.sync if gi % 2 == 0 else nc.scalar
                    eng.dma_start(
                        out=est[:, 0:jg, :].rearrange("p j k -> p (j k)"),
                        in_=est_d[:, c0 * EW:(c0 + jg) * EW])
                    xg = wk.tile([128, jgmax, HID], F32, name="xg",
                                 bufs=3)
                    for jj in range(jg):
                        nc.gpsimd.indirect_dma_start(
                            out=xg[:, jj, :], out_offset=None,
                            in_=xfull[:, :],
                            in_offset=bass.IndirectOffsetOnAxis(
                                ap=sidx_sb[:, c0 + jj:c0 + jj + 1], axis=0))
                    # z = x[src] + eh (into est's eh half)
                    nc.vector.tensor_tensor(out=est[:, 0:jg, 0:HID],
                                            in0=xg[:, 0:jg, :],
                                            in1=est[:, 0:jg, 0:HID], op=ADD)
                    # m = relu(z) in place (DVE 4x)
                    nc.vector.tensor_scalar_max(out=est[:, 0:jg, 0:HID],
                                                in0=est[:, 0:jg, 0:HID],
                                                scalar1=0.0)
                    em = wk.tile([128, jgmax, 2 * HID], BF16, name="em",
                                 bufs=3)
                    nc.scalar.activation(out=em[:, 0:jg, 0:HID],
                                         in_=est[:, 0:jg, 0:HID],
                                         func=mybir.ActivationFunctionType.Exp,
                                         scale=beta)
                    nc.vector.tensor_tensor(out=em[:, 0:jg, HID:2 * HID],
                                            in0=est[:, 0:jg, 0:HID],
                                            in1=em[:, 0:jg, 0:HID], op=MUL)
                    # segment sums on PE: one-hot matmuls, accumulate per block
                    cloc = 0
                    for bi in range(nb):
                        psb = pseg.tile([128, 2 * HID], F32, name="psb")
                        for ci in range(cs[bi]):
                            nc.tensor.matmul(out=psb[:],
                                             lhsT=est[:, cloc, HID:EW],
                                             rhs=em[:, cloc, :],
                                             start=(ci == 0),
                                             stop=(ci == cs[bi] - 1))
                            cloc += 1
                        nc.vector.tensor_copy(out=sn[:, b0 + bi, :],
                                               in_=psb[:])
                    c0 += jg
                    # epilogue for this group: y = num/max(den,.5) + eps + x
                    den = _sn_half(0, b0, nb)
                    num = _sn_half(HID, b0, nb)
                    nc.vector.tensor_scalar_max(out=den, in0=den, scalar1=0.5)
                    nc.vector.reciprocal(out=den, in_=den)
                    nc.vector.tensor_tensor(out=num, in0=num, in1=den, op=MUL)
                    nc.vector.tensor_scalar_add(out=num, in0=num,
                                                scalar1=EPS_MSG)
                    nc.vector.tensor_tensor(out=num, in0=num,
                                            in1=xbf[:, b0:b0 + nb, :], op=ADD)
                    for bi in range(nb):
                        mlp_block(b0 + bi)

            # ================= final head =================
            batched_ln_relu(h_sb, bias_sb["ng"][:, 0, :],
                            bias_sb["nb"][:, 0, :], u_ap)
            OB = 14
            for ob0 in range(0, NBLK, OB):
                onb = min(OB, NBLK - ob0)
                oc = wk.tile([128, OB, OUT_DIM], F32, name="oc")
                for i in range(onb):
                    b = ob0 + i
                    pxt = pmlp.tile([HID, 128], F32, name="pxt", tag="ptr",
                                    bufs=2)
                    nc.tensor.transpose(out=pxt[:], in_=sn[:, b, HID:2 * HID],
                                        identity=ident[:])
                    xt = nop.tile([HID, 128], F32, name="xt")
                    nc.scalar.activation(out=xt[:], in_=pxt[:],
                                         func=mybir.ActivationFunctionType.Copy)
                    po = pmlp.tile([128, OUT_DIM], F32, name="po", tag="pmm",
                                   bufs=2)
                    nc.tensor.matmul(out=po[:], lhsT=xt[:], rhs=wout_sb[:],
                                     start=True, stop=True)
                    nc.vector.tensor_tensor(out=oc[:, i, :], in0=po[:],
                                            in1=bias_sb["bout"][:],
                                            op=ADD)
                nc.sync.dma_start(
                    out=out_d[ob0 * 128:(ob0 + onb) * 128, :].rearrange(
                        "(b p) k -> p b k", p=128),
                    in_=oc[:, 0:onb, :])

    _install_wait_split(nc)
    wb_np = {k[1:-3]: v for k, v in wb_d.items() if k.startswith("_")}
    return nc, wb_np


def prepare(inputs, n_layers=L_LAYERS):
    """Returns (nc, in_maps) ready for run_bass_kernel_spmd."""
    node_feats = np.asarray(inputs["node_feats"], np.float32)
    edge_feats = np.asarray(inputs["edge_feats"], np.float32)
    src = np.asarray(inputs["src"], np.int32)
    dst = np.asarray(inputs["dst"], np.int32)
    cores, groups, ncht = host_prep(
        node_feats, edge_feats, src, dst,
        np.asarray(inputs["W_node"], np.float32),
        np.asarray(inputs["b_node"], np.float32),
        np.asarray(inputs["W_edge"], np.float32),
        np.asarray(inputs["b_edge"], np.float32))
    weights = tuple(np.asarray(inputs[k], np.float32) for k in
                    ["betas", "W1", "b1", "ln1_g", "ln1_b", "W2", "b2",
                     "norm_g", "norm_b", "W_out", "b_out"])
    nc, wb_np = build_bass(weights, groups, ncht, n_layers=n_layers)
    in_maps = []
    for k in range(N_CORES):
        c = cores[k]
        m = dict(h0=np.ascontiguousarray(c["h0"].reshape(128, NBLK * HID)),
                 est=np.ascontiguousarray(
                     c["est"].reshape(128, ncht * (HID + 128))),
                 sidx=c["sidx"])
        m.update(wb_np)
        in_maps.append(m)
    return nc, in_maps


def kernel(node_feats, edge_feats, src, dst, W_node, b_node, W_edge, b_edge,
           betas, W1, b1, ln1_g, ln1_b, W2, b2, norm_g, norm_b, W_out, b_out,
           n_layers=L_LAYERS):
    inputs = dict(node_feats=node_feats, edge_feats=edge_feats, src=src,
                  dst=dst, W_node=W_node, b_node=b_node, W_edge=W_edge,
                  b_edge=b_edge, betas=betas, W1=W1, b1=b1, ln1_g=ln1_g,
                  ln1_b=ln1_b, W2=W2, b2=b2, norm_g=norm_g, norm_b=norm_b,
                  W_out=W_out, b_out=b_out)
    nc, in_maps = prepare(inputs, n_layers=n_layers)
    res = run_bass_kernel_spmd(nc, in_maps, core_ids=list(range(N_CORES)))

    out = np.zeros((N_NODES, OUT_DIM), np.float32)
    for k in range(N_CORES):
        o = res.results[k]["out"]
        out[k * N_PER_CORE:(k + 1) * N_PER_CORE] = o[:N_PER_CORE]
    return out



# revision 3
# speedup vs baseline: 1.3691x; 1.3691x over previous
"""DeeperGCN forward on 8 TRN2 NeuronCores (Bass/Tile).

Sharding: dst-partitioned graph parallel. Core k owns original nodes
[12500k, 12500(k+1)), padded to 12544 = 98 blocks of 128. Edges live on the
core owning their dst, laid densely into 128-edge chunks per dst-block. Per
layer:
  x = relu(LN(h)) (batched big-tile ops, bf16) -> AllGather x -> batched
  indirect gather of x[src] (one DMA per ~32-chunk group) -> messages
  m = relu(x[src]+eh), e = exp(beta*m) -> per-dst per-feature softmax sums
  via one-hot matmuls accumulated in PSUM (PE does the segment reduction;
  [e|m*e] packed in one 128-col rhs) -> y = num/den + x -> GENConv MLP per
  block -> h += y.
Node/edge encoders (h0, eh) and the one-hot matrices are computed host-side
and uploaded. Pad slots carry eh = -1e9 (message exactly 0) and a zero
one-hot column, so they contribute nothing.
Softmax uses no segment-max: logits are bounded (LN output <= sqrt(63)), so
exp() cannot overflow, and e/s is shift-invariant so results match the
reference to fp rounding.
"""
import json
import numpy as np
import ml_dtypes

import concourse.bass as bass
import concourse.mybir as mybir
import concourse.tile as tile
from concourse.bass_types import AP
from concourse.bass_utils import run_bass_kernel_spmd
from concourse.masks import make_identity

# ---- problem constants (hardcoded per contract) ----
N_NODES = 100000
N_EDGES = 1200000
NODE_DIM = 128
EDGE_DIM = 8
HID = 64
OUT_DIM = 112
L_LAYERS = 7
EPS_MSG = 1e-7
LN_EPS = 1e-5

N_CORES = 8
N_PER_CORE = 12500          # original nodes per core
NBLK = 98                   # dst blocks per core (128 dsts each)
NLOC = NBLK * 128           # 12544 padded local nodes
NFULL = N_CORES * NLOC      # padded global rows in gathered x
JMAX = 24                   # max chunks per gather group
F32 = mybir.dt.float32
BF16 = mybir.dt.bfloat16
I32 = mybir.dt.int32
BIGNEG = -1e9


def _split_multi_waits(bir_bytes: bytes) -> bytes:
    """Walrus in this container allows only ONE semaphore wait per
    instruction: hoist extra waits onto same-engine NoOps."""
    d = json.loads(bir_bytes)
    ctr = 0
    for f in d["functions"]:
        for blk in f["blocks"]:
            insts = blk["instructions"]
            out = []
            changed = False
            for inst in insts:
                si = inst.get("sync_info")
                if si:
                    waits = si.get("on_wait") or []
                    if len(waits) > 1:
                        changed = True
                        for w in waits[:-1]:
                            ctr += 1
                            out.append({
                                "debug": inst.get("debug", 0),
                                "engine": inst["engine"],
                                "ins": [], "outs": [],
                                "name": f"I-wsplit-{ctr}",
                                "opcode": "NoOp",
                                "sync_info": {"on_wait": [w], "on_update": []},
                            })
                        si["on_wait"] = waits[-1:]
                out.append(inst)
            if changed:
                blk["instructions"] = out
    return json.dumps(d).encode()


def _install_wait_split(nc):
    orig = nc.to_json_bytes
    nc.to_json_bytes = lambda: _split_multi_waits(orig())


def _bcast_mid(ap, n):
    """[128, D] AP -> [128, n(bcast), D] with 0-stride middle axis."""
    return AP(ap.tensor, ap.offset, [list(ap.ap[0]), [0, n], list(ap.ap[1])])


# ---------------------------------------------------------------- host prep
def host_prep(node_feats, edge_feats, src, dst, W_node, b_node, W_edge, b_edge):
    """Per-core dense edge-chunk layout + host-side encoders + one-hots."""
    h0v = node_feats.astype(np.float32) @ W_node + b_node          # [N, HID]
    ehv = edge_feats.astype(np.float32) @ W_edge + b_edge          # [E, HID]

    owner = dst // N_PER_CORE
    cores = []
    for k in range(N_CORES):
        sel = np.nonzero(owner == k)[0]
        dl = dst[sel] - k * N_PER_CORE
        blk = dl // 128
        eidx = np.argsort(blk, kind="stable")
        cores.append(dict(sel=sel[eidx], dl=dl[eidx], blks=blk[eidx],
                          counts=np.bincount(blk, minlength=NBLK)))

    caps = np.max(np.stack([(c["counts"] + 127) // 128 for c in cores]),
                  axis=0)                               # [NBLK] chunks/block
    caps = np.maximum(caps, 1)
    # group consecutive blocks: sum of caps <= JMAX per group
    groups = []                                         # (b0, nb, [caps...])
    b0 = 0
    while b0 < NBLK:
        nb, tot = 0, 0
        while b0 + nb < NBLK and (nb == 0 or tot + caps[b0 + nb] <= JMAX):
            tot += caps[b0 + nb]
            nb += 1
        groups.append((b0, nb, [int(caps[b0 + i]) for i in range(nb)]))
        b0 += nb
    start_chunk = np.zeros(NBLK + 1, np.int64)
    start_chunk[1:] = np.cumsum(caps)
    ncht = int(start_chunk[-1])
    ncht_pad = ncht + JMAX

    for k, c in enumerate(cores):
        sel, dl, blks = c["sel"], c["dl"], c["blks"]
        starts = np.zeros(NBLK, np.int64)
        starts[1:] = np.cumsum(c["counts"])[:-1]
        ei = np.arange(len(blks)) - starts[blks]        # index within block
        cinb = ei // 128
        p = ei % 128
        chunk = start_chunk[blks] + cinb
        drow = dl % 128

        est = np.zeros((128, ncht, HID + 128), ml_dtypes.bfloat16)
        est[:, :, 0:HID] = ml_dtypes.bfloat16(BIGNEG)
        est[p, chunk, 0:HID] = ehv[sel].astype(ml_dtypes.bfloat16)
        est[p, chunk, HID + drow] = 1.0
        s = src[sel]
        sidx = np.zeros((128, ncht_pad), np.int32)
        sidx[p, chunk] = ((s // N_PER_CORE) * NLOC +
                          (s % N_PER_CORE)).astype(np.int32)
        h0 = np.zeros((NLOC, HID), np.float32)
        h0[:N_PER_CORE] = h0v[k * N_PER_CORE:(k + 1) * N_PER_CORE]
        c.clear()
        c.update(est=est, sidx=sidx,
                 h0=np.ascontiguousarray(
                     h0.reshape(NBLK, 128, HID).transpose(1, 0, 2)))
    return cores, groups, ncht


# ---------------------------------------------------------------- device build
def build_bass(weights, groups, ncht, n_layers=L_LAYERS):
    (betas, W1, b1, ln1_g, ln1_b, W2, b2, norm_g, norm_b, W_out, b_out) = weights
    nc = bass.Bass("TRN2", target_bir_lowering=False, debug=False,
                   num_devices=N_CORES)

    h0_d = nc.dram_tensor("h0", [128, NBLK * HID], F32, kind="ExternalInput")
    est_d = nc.dram_tensor("est", [128, ncht * (HID + 128)], BF16,
                           kind="ExternalInput")
    sidx_d = nc.dram_tensor("sidx", [128, ncht + JMAX], I32,
                            kind="ExternalInput")
    wb_d = {}
    for name, arr in [
        ("w1", W1.transpose(1, 0, 2)),                 # [HID, L, 2H]
        ("w2", W2.transpose(1, 0, 2)),                 # [2H, L, HID]
        ("wout", W_out),
        ("b1", np.broadcast_to(b1[:, None, :], (L_LAYERS, 128, 2 * HID)).transpose(1, 0, 2)),
        ("l1g", np.broadcast_to(ln1_g[:, None, :], (L_LAYERS, 128, 2 * HID)).transpose(1, 0, 2)),
        ("l1b", np.broadcast_to(ln1_b[:, None, :], (L_LAYERS, 128, 2 * HID)).transpose(1, 0, 2)),
        ("b2", np.broadcast_to(b2[:, None, :], (L_LAYERS, 128, HID)).transpose(1, 0, 2)),
        ("ng", np.broadcast_to(norm_g[:, None, :], (L_LAYERS, 128, HID)).transpose(1, 0, 2)),
        ("nb", np.broadcast_to(norm_b[:, None, :], (L_LAYERS, 128, HID)).transpose(1, 0, 2)),
        ("bout", np.broadcast_to(b_out, (128, OUT_DIM))),
    ]:
        a = np.ascontiguousarray(arr, np.float32)
        wb_d[name] = nc.dram_tensor(name, list(a.shape), F32, kind="ExternalInput")
        wb_d[f"_{name}_np"] = a
    out_d = nc.dram_tensor("out", [NLOC, OUT_DIM], F32, kind="ExternalOutput")

    jgmax = max(sum(cs) for _, _, cs in groups)

    with tile.TileContext(nc) as tc:
        with tc.tile_pool(name="persist", bufs=1) as pp, \
             tc.tile_pool(name="dram", bufs=1, space="DRAM") as dram, \
             tc.tile_pool(name="work", bufs=2) as wk, \
             tc.tile_pool(name="nodeops", bufs=3) as nop, \
             tc.tile_pool(name="pseg", bufs=4, space="PSUM") as pseg, \
             tc.tile_pool(name="pmlp", bufs=1, space="PSUM") as pmlp:

            # ---- persistent SBUF state ----
            h_sb = pp.tile([128, NBLK, HID], F32)
            nc.sync.dma_start(out=h_sb[:].rearrange("p b k -> p (b k)"),
                              in_=h0_d[:, :])
            xbf = pp.tile([128, NBLK, HID], BF16)
            # sn: [p, b, 0:HID]=den (also LN scratch sq);
            #     [p, b, HID:2H]=num -> y (also LN scratch u)
            sn = pp.tile([128, NBLK, 2 * HID], F32)
            sidx_sb = pp.tile([128, ncht + JMAX], I32)
            nc.sync.dma_start(out=sidx_sb[:], in_=sidx_d[:, :])
            ident = pp.tile([128, 128], F32)
            make_identity(nc, ident[:])
            epsq_sb = pp.tile([128, 1], F32, name="epsq")
            nc.vector.memset(epsq_sb[:], float(HID) * HID * LN_EPS)
            eps_sb = pp.tile([128, 1], F32, name="eps1")
            nc.vector.memset(eps_sb[:], LN_EPS)

            w1_sb = pp.tile([HID, L_LAYERS, 2 * HID], F32)
            nc.sync.dma_start(out=w1_sb[:], in_=wb_d["w1"][:, :, :])
            w2_sb = pp.tile([2 * HID, L_LAYERS, HID], F32)
            nc.sync.dma_start(out=w2_sb[:], in_=wb_d["w2"][:, :, :])
            wout_sb = pp.tile([HID, OUT_DIM], F32)
            nc.sync.dma_start(out=wout_sb[:], in_=wb_d["wout"][:, :])
            bias_sb = {}
            for nm, dd in [("b1", 2 * HID), ("l1g", 2 * HID), ("l1b", 2 * HID),
                           ("b2", HID), ("ng", HID), ("nb", HID)]:
                bias_sb[nm] = pp.tile([128, L_LAYERS, dd], F32, name=f"bs_{nm}")
                nc.sync.dma_start(out=bias_sb[nm][:], in_=wb_d[nm][:, :, :])
            bias_sb["bout"] = pp.tile([128, OUT_DIM], F32, name="bs_bout")
            nc.sync.dma_start(out=bias_sb["bout"][:], in_=wb_d["bout"][:, :])

            # ---- DRAM internals: collective buffers, one pair per layer ----
            xins = [dram.tile([NLOC, HID], F32, name=f"xin{i}")
                    for i in range(n_layers)]
            xfulls = [dram.tile([NFULL, HID], F32, addr_space="Shared",
                                name=f"xfull{i}") for i in range(n_layers)]

            D = float(HID)
            ADD = mybir.AluOpType.add
            MUL = mybir.AluOpType.mult
            SUB = mybir.AluOpType.subtract

            def _sn_half(off, b0=0, nb=NBLK):
                full = sn[:]
                return AP(full.tensor, full.offset + off + b0 * 2 * HID,
                          [list(full.ap[0]), [2 * HID, nb], [1, HID]])

            sq_ap = _sn_half(0)          # den half as LN scratch
            u_ap = _sn_half(HID)         # num half as LN scratch

            def batched_ln_relu(src_tile, g_ap, b_ap, out_ap):
                """out = relu(LN(src) * g + b), batched over all NBLK blocks.
                src [128, NBLK, HID] f32; g/b [128, HID] (per-feature).
                Uses sn as scratch (sq then u)."""
                s1 = nop.tile([128, NBLK], F32, name="ln_s1")
                nc.vector.tensor_reduce(out=s1[:], in_=src_tile[:],
                                        axis=mybir.AxisListType.X, op=ADD)
                nc.vector.tensor_tensor(out=sq_ap, in0=src_tile[:],
                                        in1=src_tile[:], op=MUL)
                s2 = nop.tile([128, NBLK], F32, name="ln_s2")
                nc.vector.tensor_reduce(out=s2[:], in_=sq_ap,
                                        axis=mybir.AxisListType.X, op=ADD)
                t1 = nop.tile([128, NBLK], F32, name="ln_t1")
                nc.vector.tensor_tensor(out=t1[:], in0=s1[:], in1=s1[:],
                                        op=MUL)
                # q = D*S2 - S1^2  (= D^2 * var)
                q = nop.tile([128, NBLK], F32, name="ln_q")
                nc.vector.scalar_tensor_tensor(out=q[:], in0=s2[:], scalar=D,
                                               in1=t1[:], op0=MUL, op1=SUB)
                # r = 1/sqrt(q + D^2*eps);  A = D*r;  B = S1*r
                qs = nop.tile([128, NBLK], F32, name="ln_qs")
                nc.scalar.activation(out=qs[:], in_=q[:],
                                     func=mybir.ActivationFunctionType.Sqrt,
                                     bias=epsq_sb[:])
                r = nop.tile([128, NBLK], F32, name="ln_r")
                nc.vector.reciprocal(out=r[:], in_=qs[:])
                a = nop.tile([128, NBLK], F32, name="ln_a")
                nc.vector.tensor_scalar_mul(out=a[:], in0=r[:], scalar1=D)
                bb = nop.tile([128, NBLK], F32, name="ln_b")
                nc.vector.tensor_tensor(out=bb[:], in0=s1[:], in1=r[:],
                                        op=MUL)
                # u = h*A - B ; v = u*g + b ; out = relu(v)
                nc.vector.tensor_tensor(out=u_ap, in0=src_tile[:],
                                        in1=a[:].broadcast_to([128, NBLK, HID]),
                                        op=MUL)
                nc.vector.tensor_tensor(out=u_ap, in0=u_ap,
                                        in1=bb[:].broadcast_to([128, NBLK, HID]),
                                        op=SUB)
                nc.vector.tensor_tensor(out=u_ap, in0=u_ap,
                                        in1=_bcast_mid(g_ap, NBLK), op=MUL)
                nc.vector.tensor_tensor(out=u_ap, in0=u_ap,
                                        in1=_bcast_mid(b_ap, NBLK), op=ADD)
                nc.scalar.activation(out=out_ap, in_=u_ap,
                                     func=mybir.ActivationFunctionType.Relu)

            # ================= layers =================
            for li in range(n_layers):
                beta = float(betas[li])
                xin, xfull = xins[li], xfulls[li]

                # ---- x = relu(LN(h)) ----
                batched_ln_relu(h_sb, bias_sb["ng"][:, li, :],
                                bias_sb["nb"][:, li, :], u_ap)
                nc.sync.dma_start(
                    out=xin[:, :].rearrange("(b p) k -> p b k", p=128),
                    in_=u_ap)
                nc.vector.tensor_copy(out=xbf[:], in_=u_ap)
                # ---- all-gather x ----
                nc.gpsimd.collective_compute(
                    "AllGather", mybir.AluOpType.bypass,
                    replica_groups=[list(range(N_CORES))],
                    ins=[xin[:, :].opt()], outs=[xfull[:, :].opt()])

                # ---- MLP per block: h += W2 @ relu(LN1(y@W1+b1)) + b2 ----
                def mlp_block(b):
                    pyt = pmlp.tile([HID, 128], F32, name="pyt", tag="ptr",
                                    bufs=2)
                    nc.tensor.transpose(out=pyt[:], in_=sn[:, b, HID:2 * HID],
                                        identity=ident[:])
                    yt = nop.tile([HID, 128], F32, name="yt")
                    nc.scalar.activation(out=yt[:], in_=pyt[:],
                                         func=mybir.ActivationFunctionType.Copy)
                    pm1 = pmlp.tile([128, 2 * HID], F32, name="pm1", tag="pmm",
                                    bufs=2)
                    nc.tensor.matmul(out=pm1[:], lhsT=yt[:],
                                     rhs=w1_sb[:, li, :], start=True, stop=True)
                    z0 = nop.tile([128, 2 * HID], F32, name="z0")
                    nc.vector.tensor_tensor(out=z0[:], in0=pm1[:],
                                            in1=bias_sb["b1"][:, li, :],
                                            op=ADD)
                    st1 = nop.tile([128, 6], F32, name="st1")
                    nc.vector.bn_stats(out=st1[:], in_=z0[:])
                    mv1 = nop.tile([128, 2], F32, name="mv1")
                    nc.vector.bn_aggr(out=mv1[:], in_=st1[:])
                    sd = nop.tile([128, 1], F32, name="sd")
                    nc.scalar.activation(out=sd[:], in_=mv1[:, 1:2],
                                         func=mybir.ActivationFunctionType.Sqrt,
                                         bias=eps_sb[:])
                    rstd = nop.tile([128, 1], F32, name="rstd")
                    nc.vector.reciprocal(out=rstd[:], in_=sd[:])
                    z1 = nop.tile([128, 2 * HID], F32, name="z1")
                    nc.vector.scalar_tensor_tensor(
                        out=z1[:], in0=z0[:], scalar=mv1[:, 0:1],
                        in1=rstd[:].broadcast_to([128, 2 * HID]),
                        op0=SUB, op1=MUL)
                    nc.gpsimd.tensor_tensor(out=z1[:], in0=z1[:],
                                            in1=bias_sb["l1g"][:, li, :],
                                            op=MUL)
                    nc.gpsimd.tensor_tensor(out=z1[:], in0=z1[:],
                                            in1=bias_sb["l1b"][:, li, :],
                                            op=ADD)
                    nc.scalar.activation(out=z1[:], in_=z1[:],
                                         func=mybir.ActivationFunctionType.Relu)
                    pzt = pmlp.tile([128, 128], F32, name="pzt", tag="ptr",
                                    bufs=2)
                    nc.tensor.transpose(out=pzt[:], in_=z1[:], identity=ident[:])
                    zt = nop.tile([128, 128], F32, name="zt")
                    nc.scalar.activation(out=zt[:], in_=pzt[:],
                                         func=mybir.ActivationFunctionType.Copy)
                    pm2 = pmlp.tile([128, HID], F32, name="pm2", tag="pmm",
                                    bufs=2)
                    nc.tensor.matmul(out=pm2[:], lhsT=zt[:],
                                     rhs=w2_sb[:, li, :], start=True, stop=True)
                    y2 = nop.tile([128, HID], F32, name="y2")
                    nc.vector.tensor_tensor(out=y2[:], in0=pm2[:],
                                            in1=bias_sb["b2"][:, li, :],
                                            op=ADD)
                    nc.gpsimd.tensor_tensor(out=h_sb[:, b, :],
                                            in0=h_sb[:, b, :], in1=y2[:],
                                            op=ADD)


                # ---- edge phase: per group ----
                c0 = 0
                EW = HID + 128
                for gi, (b0, nb, cs) in enumerate(groups):
                    jg = sum(cs)
                    est = wk.tile([128, jgmax, EW], BF16, name="est", bufs=3)
                    eng = nc.sync if gi % 2 == 0 else nc.scalar
                    eng.dma_start(
                        out=est[:, 0:jg, :].rearrange("p j k -> p (j k)"),
                        in_=est_d[:, c0 * EW:(c0 + jg) * EW])
                    xg = wk.tile([128, jgmax, HID], F32, name="xg",
                                 bufs=3)
                    for jj in range(jg):
                        nc.gpsimd.indirect_dma_start(
                            out=xg[:, jj, :], out_offset=None,
                            in_=xfull[:, :],
                            in_offset=bass.IndirectOffsetOnAxis(
                                ap=sidx_sb[:, c0 + jj:c0 + jj + 1], axis=0))
                    # z = x[src] + eh (into est's eh half)
                    nc.vector.tensor_tensor(out=est[:, 0:jg, 0:HID],
                                            in0=xg[:, 0:jg, :],
                                            in1=est[:, 0:jg, 0:HID], op=ADD)
                    # m = relu(z) in place (DVE 4x)
                    nc.vector.tensor_scalar_max(out=est[:, 0:jg, 0:HID],
                                                in0=est[:, 0:jg, 0:HID],
                                                scalar1=0.0)
                    em = wk.tile([128, jgmax, 2 * HID], BF16, name="em",
                                 bufs=3)
                    nc.scalar.activation(out=em[:, 0:jg, 0:HID],
                                         in_=est[:, 0:jg, 0:HID],
                                         func=mybir.ActivationFunctionType.Exp,
                                         scale=beta)
                    nc.vector.tensor_tensor(out=em[:, 0:jg, HID:2 * HID],
                                            in0=est[:, 0:jg, 0:HID],
                                            in1=em[:, 0:jg, 0:HID], op=MUL)
                    # segment sums on PE: one-hot matmuls, accumulate per block
                    cloc = 0
                    for bi in range(nb):
                        psb = pseg.tile([128, 2 * HID], F32, name="psb")
                        for ci in range(cs[bi]):
                            nc.tensor.matmul(out=psb[:],
                                             lhsT=est[:, cloc, HID:EW],
                                             rhs=em[:, cloc, :],
                                             start=(ci == 0),
                                             stop=(ci == cs[bi] - 1))
                            cloc += 1
                        nc.vector.tensor_copy(out=sn[:, b0 + bi, :],
                                               in_=psb[:])
                    c0 += jg
                    # epilogue for this group: y = num/max(den,.5) + eps + x
                    den = _sn_half(0, b0, nb)
                    num = _sn_half(HID, b0, nb)
                    nc.vector.tensor_scalar_max(out=den, in0=den, scalar1=0.5)
                    nc.vector.reciprocal(out=den, in_=den)
                    nc.vector.tensor_tensor(out=num, in0=num, in1=den, op=MUL)
                    nc.vector.tensor_scalar_add(out=num, in0=num,
                                                scalar1=EPS_MSG)
                    nc.vector.tensor_tensor(out=num, in0=num,
                                            in1=xbf[:, b0:b0 + nb, :], op=ADD)
                    for bi in range(nb):
                        mlp_block(b0 + bi)

            # ================= final head =================
            batched_ln_relu(h_sb, bias_sb["ng"][:, 0, :],
                            bias_sb["nb"][:, 0, :], u_ap)
            OB = 14
            for ob0 in range(0, NBLK, OB):
                onb = min(OB, NBLK - ob0)
                oc = wk.tile([128, OB, OUT_DIM], F32, name="oc")
                for i in range(onb):
                    b = ob0 + i
                    pxt = pmlp.tile([HID, 128], F32, name="pxt", tag="ptr",
                                    bufs=2)
                    nc.tensor.transpose(out=pxt[:], in_=sn[:, b, HID:2 * HID],
                                        identity=ident[:])
                    xt = nop.tile([HID, 128], F32, name="xt")
                    nc.scalar.activation(out=xt[:], in_=pxt[:],
                                         func=mybir.ActivationFunctionType.Copy)
                    po = pmlp.tile([128, OUT_DIM], F32, name="po", tag="pmm",
                                   bufs=2)
                    nc.tensor.matmul(out=po[:], lhsT=xt[:], rhs=wout_sb[:],
                                     start=True, stop=True)
                    nc.vector.tensor_tensor(out=oc[:, i, :], in0=po[:],
                                            in1=bias_sb["bout"][:],
                                            op=ADD)
                nc.sync.dma_start(
                    out=out_d[ob0 * 128:(ob0 + onb) * 128, :].rearrange(
                        "(b p) k -> p b k", p=128),
                    in_=oc[:, 0:onb, :])

    _install_wait_split(nc)
    wb_np = {k[1:-3]: v for k, v in wb_d.items() if k.startswith("_")}
    return nc, wb_np


def prepare(inputs, n_layers=L_LAYERS):
    """Returns (nc, in_maps) ready for run_bass_kernel_spmd."""
    node_feats = np.asarray(inputs["node_feats"], np.float32)
    edge_feats = np.asarray(inputs["edge_feats"], np.float32)
    src = np.asarray(inputs["src"], np.int32)
    dst = np.asarray(inputs["dst"], np.int32)
    cores, groups, ncht = host_prep(
        node_feats, edge_feats, src, dst,
        np.asarray(inputs["W_node"], np.float32),
        np.asarray(inputs["b_node"], np.float32),
        np.asarray(inputs["W_edge"], np.float32),
        np.asarray(inputs["b_edge"], np.float32))
    weights = tuple(np.asarray(inputs[k], np.float32) for k in
                    ["betas", "W1", "b1", "ln1_g", "ln1_b", "W2", "b2",
                     "norm_g", "norm_b", "W_out", "b_out"])
    nc, wb_np = build_bass(weights, groups, ncht, n_layers=n_layers)
    in_maps = []
    for k in range(N_CORES):
        c = cores[k]
        m = dict(h0=np.ascontiguousarray(c["h0"].reshape(128, NBLK * HID)),
                 est=np.ascontiguousarray(
                     c["est"].reshape(128, ncht * (HID + 128))),
                 sidx=c["sidx"])
        m.update(wb_np)
        in_maps.append(m)
    return nc, in_maps


def kernel(node_feats, edge_feats, src, dst, W_node, b_node, W_edge, b_edge,
           betas, W1, b1, ln1_g, ln1_b, W2, b2, norm_g, norm_b, W_out, b_out,
           n_layers=L_LAYERS):
    inputs = dict(node_feats=node_feats, edge_feats=edge_feats, src=src,
                  dst=dst, W_node=W_node, b_node=b_node, W_edge=W_edge,
                  b_edge=b_edge, betas=betas, W1=W1, b1=b1, ln1_g=ln1_g,
                  ln1_b=ln1_b, W2=W2, b2=b2, norm_g=norm_g, norm_b=norm_b,
                  W_out=W_out, b_out=b_out)
    nc, in_maps = prepare(inputs, n_layers=n_layers)
    res = run_bass_kernel_spmd(nc, in_maps, core_ids=list(range(N_CORES)))

    out = np.zeros((N_NODES, OUT_DIM), np.float32)
    for k in range(N_CORES):
        o = res.results[k]["out"]
        out[k * N_PER_CORE:(k + 1) * N_PER_CORE] = o[:N_PER_CORE]
    return out



# revision 5
# speedup vs baseline: 2.0863x; 1.5238x over previous
"""DeeperGCN forward on 8 TRN2 NeuronCores (Bass/Tile).

Sharding: dst-partitioned graph parallel. Core k owns original nodes
[12500k, 12500(k+1)), padded to 12544 = 98 blocks of 128. Edges live on the
core owning their dst, laid densely into 128-edge chunks per (dst-block,
src-shard); within a block edges are split into two runs by which half of
the padded global node space their src falls in, so gathers can use int16
indices. Per layer:
  x = relu(LN(h)) (batched big-tile ops) -> AllGather x (bf16, viewed as
  50176 node-pair rows of 256B) -> batched dma_gather of x pair-rows per
  8-chunk piece (mlp-library Q7 gather, int16 shard-local indices
  replicated across the 8 16-partition groups) -> parity-select the
  even/odd 64-col half on DVE -> messages m = relu(x[src]+eh),
  e = exp(beta*m) -> per-dst per-feature softmax sums via one-hot matmuls
  accumulated in PSUM ([e|m*e] packed in one 128-col rhs) ->
  y = num/den + x -> GENConv MLP per block -> h += y.
Node/edge encoders (h0, eh) and the one-hot matrices are computed host-side
and uploaded. Pad slots carry eh = -1e9 (message exactly 0), idx 0 and a
zero one-hot column, so they contribute nothing.
Softmax uses no segment-max: logits are bounded (LN output <= sqrt(63)), so
exp() cannot overflow, and e/s is shift-invariant so results match the
reference to fp rounding.
"""
import json
import numpy as np
import ml_dtypes

import concourse.bass as bass
import concourse.mybir as mybir
import concourse.tile as tile
from concourse.bass_types import AP
from concourse.bass_utils import run_bass_kernel_spmd
from concourse.masks import make_identity
from concourse import library_config
from concourse.library_overlay import lower_extended_insts

# ---- problem constants (hardcoded per contract) ----
N_NODES = 100000
N_EDGES = 1200000
NODE_DIM = 128
EDGE_DIM = 8
HID = 64
OUT_DIM = 112
L_LAYERS = 7
EPS_MSG = 1e-7
LN_EPS = 1e-5

N_CORES = 8
N_PER_CORE = 12500          # original nodes per core
NBLK = 98                   # dst blocks per core (128 dsts each)
NLOC = NBLK * 128            # 12544 padded local nodes
NFULL = N_CORES * NLOC       # padded global rows in gathered x
NPAIR = NFULL // 2           # 50176 node-pair rows (256B bf16 each)
SHARD = NPAIR // 2           # 25088 pair rows per index shard (int16-safe)
HBLK = 49                    # blocks per half-graph AllGather piece
HPAIR = HBLK * 64            # 3136 local pair rows per half
JMAX = 24                    # max chunks per est group
PIECE = 8                    # chunks per dma_gather (1024 idxs = ring cap)
F32 = mybir.dt.float32
BF16 = mybir.dt.bfloat16
I16 = mybir.dt.int16
BIGNEG = -1e9
EW = HID + 128               # est row: [eh64 | onehot128]


def _split_multi_waits(bir_bytes: bytes) -> bytes:
    """Walrus in this container allows only ONE semaphore wait per
    instruction: hoist extra waits onto same-engine NoOps."""
    d = json.loads(bir_bytes)
    ctr = 0
    for f in d["functions"]:
        for blk in f["blocks"]:
            insts = blk["instructions"]
            out = []
            changed = False
            for inst in insts:
                si = inst.get("sync_info")
                if si:
                    waits = si.get("on_wait") or []
                    if len(waits) > 1:
                        changed = True
                        for w in waits[:-1]:
                            ctr += 1
                            out.append({
                                "debug": inst.get("debug", 0),
                                "engine": inst["engine"],
                                "ins": [], "outs": [],
                                "name": f"I-wsplit-{ctr}",
                                "opcode": "NoOp",
                                "sync_info": {"on_wait": [w], "on_update": []},
                            })
                        si["on_wait"] = waits[-1:]
                out.append(inst)
            if changed:
                blk["instructions"] = out
    return json.dumps(d).encode()


def _install_wait_split(nc):
    orig = nc.to_json_bytes
    nc.to_json_bytes = lambda: _split_multi_waits(orig())


def _bcast_mid(ap, n):
    """[128, D] AP -> [128, n(bcast), D] with 0-stride middle axis."""
    return AP(ap.tensor, ap.offset, [list(ap.ap[0]), [0, n], list(ap.ap[1])])


# ---------------------------------------------------------------- host prep
def host_prep(node_feats, edge_feats, src, dst, W_node, b_node, W_edge, b_edge):
    """Per-core dense (block, shard)-run edge layout + host-side encoders,
    one-hots, parity planes and int16 gather indices."""
    h0v = node_feats.astype(np.float32) @ W_node + b_node          # [N, HID]
    ehv = edge_feats.astype(np.float32) @ W_edge + b_edge          # [E, HID]

    g = (src // N_PER_CORE).astype(np.int64) * NLOC + (src % N_PER_CORE)
    parity = (g & 1).astype(np.float32)
    core = g // NLOC
    lpl = (g % NLOC) >> 1                              # local pair [0, 6272)
    shard = (lpl >= HPAIR).astype(np.int64)            # which half-graph AG
    lp = core * HPAIR + (lpl % HPAIR)                  # row in shard buffer

    owner = dst // N_PER_CORE
    cores = []
    for k in range(N_CORES):
        sel = np.nonzero(owner == k)[0]
        dl = dst[sel] - k * N_PER_CORE
        blk = dl // 128
        key = blk * 2 + shard[sel]
        eidx = np.argsort(key, kind="stable")
        cores.append(dict(sel=sel[eidx], dl=dl[eidx], key=key[eidx],
                          counts=np.bincount(key, minlength=NBLK * 2)))

    caps = np.max(np.stack([(c["counts"] + 127) // 128 for c in cores]),
                  axis=0).reshape(NBLK, 2)             # [NBLK, 2] chunks
    for b in range(NBLK):
        if caps[b].sum() == 0:
            caps[b, 0] = 1

    # group consecutive blocks: sum of caps <= JMAX per group
    groups = []          # dict(b0, nb, js0, js1, c0, blocks=[(b, [pos...])])
    b0 = 0
    c0 = 0
    while b0 < NBLK:
        nb, tot = 0, 0
        while b0 + nb < NBLK and (nb == 0 or
                                  tot + caps[b0 + nb].sum() <= JMAX):
            tot += caps[b0 + nb].sum()
            nb += 1
        js0 = int(caps[b0:b0 + nb, 0].sum())
        js1 = int(caps[b0:b0 + nb, 1].sum())
        blocks = []
        off0, off1 = 0, js0
        for i in range(nb):
            pos = (list(range(off0, off0 + int(caps[b0 + i, 0]))) +
                   list(range(off1, off1 + int(caps[b0 + i, 1]))))
            blocks.append((b0 + i, pos))
            off0 += int(caps[b0 + i, 0])
            off1 += int(caps[b0 + i, 1])
        groups.append(dict(b0=b0, nb=nb, js0=js0, js1=js1, c0=c0,
                           blocks=blocks))
        c0 += js0 + js1
        b0 += nb
    ncht = c0

    # global chunk base for each (group, shard-section) start, then
    # per (block, shard) run start chunk
    run_chunk = np.zeros((NBLK, 2), np.int64)
    for grp in groups:
        off0 = grp["c0"]
        off1 = grp["c0"] + grp["js0"]
        for i in range(grp["nb"]):
            b = grp["b0"] + i
            run_chunk[b, 0] = off0
            run_chunk[b, 1] = off1
            off0 += caps[b, 0]
            off1 += caps[b, 1]

    for k, c in enumerate(cores):
        sel, dl, key = c["sel"], c["dl"], c["key"]
        counts = c["counts"]
        starts = np.zeros(NBLK * 2, np.int64)
        starts[1:] = np.cumsum(counts)[:-1]
        ei = np.arange(len(key)) - starts[key]          # index within run
        cinr = ei // 128
        p = ei % 128
        chunk = run_chunk.reshape(-1)[key] + cinr
        drow = dl % 128

        est = np.zeros((128, ncht, EW), ml_dtypes.bfloat16)
        est[:, :, 0:HID] = ml_dtypes.bfloat16(BIGNEG)
        est[p, chunk, 0:HID] = ehv[sel].astype(ml_dtypes.bfloat16)
        est[p, chunk, HID + drow] = 1.0

        par = np.zeros((128, ncht), ml_dtypes.bfloat16)
        par[p, chunk] = parity[sel].astype(ml_dtypes.bfloat16)

        idx16 = np.zeros((16, ncht * 8), np.int16)
        idx16[p % 16, chunk * 8 + p // 16] = lp[sel].astype(np.int16)
        idx16 = np.tile(idx16, (8, 1))                  # replicate 8 cores

        h0 = np.zeros((NLOC, HID), np.float32)
        h0[:N_PER_CORE] = h0v[k * N_PER_CORE:(k + 1) * N_PER_CORE]
        c.clear()
        c.update(est=est, idx16=idx16, par=par,
                 h0=np.ascontiguousarray(
                     h0.reshape(NBLK, 128, HID).transpose(1, 0, 2)))
    return cores, groups, ncht


# ---------------------------------------------------------------- device build
def build_bass(weights, groups, ncht, n_layers=L_LAYERS):
    (betas, W1, b1, ln1_g, ln1_b, W2, b2, norm_g, norm_b, W_out, b_out) = weights
    nc = bass.Bass("TRN2", target_bir_lowering=False, debug=False,
                   num_devices=N_CORES, num_swdge_queues=2)

    h0_d = nc.dram_tensor("h0", [128, NBLK * HID], F32, kind="ExternalInput")
    est_d = nc.dram_tensor("est", [128, ncht * EW], BF16,
                           kind="ExternalInput")
    idx_d = nc.dram_tensor("idx16", [128, ncht * 8], I16,
                           kind="ExternalInput")
    par_d = nc.dram_tensor("par", [128, ncht], BF16, kind="ExternalInput")
    wb_d = {}
    for name, arr in [
        ("w1", W1.transpose(1, 0, 2)),                 # [HID, L, 2H]
        ("w2", W2.transpose(1, 0, 2)),                 # [2H, L, HID]
        ("wout", W_out),
        ("b1", np.broadcast_to(b1[:, None, :], (L_LAYERS, 128, 2 * HID)).transpose(1, 0, 2)),
        ("l1g", np.broadcast_to(ln1_g[:, None, :], (L_LAYERS, 128, 2 * HID)).transpose(1, 0, 2)),
        ("l1b", np.broadcast_to(ln1_b[:, None, :], (L_LAYERS, 128, 2 * HID)).transpose(1, 0, 2)),
        ("b2", np.broadcast_to(b2[:, None, :], (L_LAYERS, 128, HID)).transpose(1, 0, 2)),
        ("ng", np.broadcast_to(norm_g[:, None, :], (L_LAYERS, 128, HID)).transpose(1, 0, 2)),
        ("nb", np.broadcast_to(norm_b[:, None, :], (L_LAYERS, 128, HID)).transpose(1, 0, 2)),
        ("bout", np.broadcast_to(b_out, (128, OUT_DIM))),
    ]:
        a = np.ascontiguousarray(arr, np.float32)
        wb_d[name] = nc.dram_tensor(name, list(a.shape), F32, kind="ExternalInput")
        wb_d[f"_{name}_np"] = a
    out_d = nc.dram_tensor("out", [NLOC, OUT_DIM], F32, kind="ExternalOutput")

    jgmax = max(g["js0"] + g["js1"] for g in groups)

    with tile.TileContext(nc) as tc:
        nc.gpsimd.load_library(library_config.mlp)
        with tc.tile_pool(name="persist", bufs=1) as pp, \
             tc.tile_pool(name="dram", bufs=1, space="DRAM") as dram, \
             tc.tile_pool(name="work", bufs=2) as wk, \
             tc.tile_pool(name="nodeops", bufs=3) as nop, \
             tc.tile_pool(name="pseg", bufs=4, space="PSUM") as pseg, \
             tc.tile_pool(name="pmlp", bufs=1, space="PSUM") as pmlp:

            # ---- persistent SBUF state ----
            h_sb = pp.tile([128, NBLK, HID], F32)
            nc.sync.dma_start(out=h_sb[:].rearrange("p b k -> p (b k)"),
                              in_=h0_d[:, :])
            xbf = pp.tile([128, NBLK, HID], BF16)
            # sn: [p, b, 0:HID]=den (also LN scratch sq);
            #     [p, b, HID:2H]=num -> y (also LN scratch u)
            sn = pp.tile([128, NBLK, 2 * HID], F32)
            par_sb = pp.tile([128, ncht], BF16)
            nc.sync.dma_start(out=par_sb[:], in_=par_d[:, :])
            ident = pp.tile([128, 128], F32)
            make_identity(nc, ident[:])
            epsq_sb = pp.tile([128, 1], F32, name="epsq")
            nc.vector.memset(epsq_sb[:], float(HID) * HID * LN_EPS)
            eps_sb = pp.tile([128, 1], F32, name="eps1")
            nc.vector.memset(eps_sb[:], LN_EPS)

            w1_sb = pp.tile([HID, L_LAYERS, 2 * HID], F32)
            nc.sync.dma_start(out=w1_sb[:], in_=wb_d["w1"][:, :, :])
            w2_sb = pp.tile([2 * HID, L_LAYERS, HID], F32)
            nc.sync.dma_start(out=w2_sb[:], in_=wb_d["w2"][:, :, :])
            wout_sb = pp.tile([HID, OUT_DIM], F32)
            nc.sync.dma_start(out=wout_sb[:], in_=wb_d["wout"][:, :])
            bias_sb = {}
            for nm, dd in [("b1", 2 * HID), ("l1g", 2 * HID), ("l1b", 2 * HID),
                           ("b2", HID), ("ng", HID), ("nb", HID)]:
                bias_sb[nm] = pp.tile([128, L_LAYERS, dd], F32, name=f"bs_{nm}")
                nc.sync.dma_start(out=bias_sb[nm][:], in_=wb_d[nm][:, :, :])
            bias_sb["bout"] = pp.tile([128, OUT_DIM], F32, name="bs_bout")
            nc.sync.dma_start(out=bias_sb["bout"][:], in_=wb_d["bout"][:, :])

            # ---- DRAM internals: collective buffers, two halves per layer ----
            xins = [[dram.tile([HBLK * 128, HID], BF16, name=f"xin{i}h{h}")
                     for h in range(2)] for i in range(n_layers)]
            xfulls = [[dram.tile([SHARD, 2 * HID], BF16, addr_space="Shared",
                                 name=f"xfull{i}h{h}") for h in range(2)]
                      for i in range(n_layers)]

            D = float(HID)
            ADD = mybir.AluOpType.add
            MUL = mybir.AluOpType.mult
            SUB = mybir.AluOpType.subtract

            qrr = [0]
            nidx_regs = {}

            def nidx_reg(n):
                if n not in nidx_regs:
                    nidx_regs[n] = nc.gpsimd.to_reg(n)
                return nidx_regs[n]

            def _sn_half(off, b0=0, nb=NBLK):
                full = sn[:]
                return AP(full.tensor, full.offset + off + b0 * 2 * HID,
                          [list(full.ap[0]), [2 * HID, nb], [1, HID]])

            sq_ap = _sn_half(0)          # den half as LN scratch
            u_ap = _sn_half(HID)         # num half as LN scratch

            def batched_ln_relu(g_ap, b_ap, b0=0, nb=NBLK):
                """sn num half [b0:b0+nb] = relu(LN(h) * g + b), batched over
                nb blocks. g/b [128, HID] (per-feature). Uses sn as scratch
                (sq then u)."""
                src_tile = h_sb[:, b0:b0 + nb, :]
                sq = _sn_half(0, b0, nb)
                u = _sn_half(HID, b0, nb)
                s1 = nop.tile([128, NBLK], F32, name="ln_s1")[:, 0:nb]
                nc.vector.tensor_reduce(out=s1, in_=src_tile,
                                        axis=mybir.AxisListType.X, op=ADD)
                nc.vector.tensor_tensor(out=sq, in0=src_tile,
                                        in1=src_tile, op=MUL)
                s2 = nop.tile([128, NBLK], F32, name="ln_s2")[:, 0:nb]
                nc.vector.tensor_reduce(out=s2, in_=sq,
                                        axis=mybir.AxisListType.X, op=ADD)
                t1 = nop.tile([128, NBLK], F32, name="ln_t1")[:, 0:nb]
                nc.vector.tensor_tensor(out=t1, in0=s1, in1=s1,
                                        op=MUL)
                # q = D*S2 - S1^2  (= D^2 * var)
                q = nop.tile([128, NBLK], F32, name="ln_q")[:, 0:nb]
                nc.vector.scalar_tensor_tensor(out=q, in0=s2, scalar=D,
                                               in1=t1, op0=MUL, op1=SUB)
                # r = 1/sqrt(q + D^2*eps);  A = D*r;  B = S1*r
                qs = nop.tile([128, NBLK], F32, name="ln_qs")[:, 0:nb]
                nc.scalar.activation(out=qs, in_=q,
                                     func=mybir.ActivationFunctionType.Sqrt,
                                     bias=epsq_sb[:])
                r = nop.tile([128, NBLK], F32, name="ln_r")[:, 0:nb]
                nc.vector.reciprocal(out=r, in_=qs)
                a = nop.tile([128, NBLK], F32, name="ln_a")[:, 0:nb]
                nc.vector.tensor_scalar_mul(out=a, in0=r, scalar1=D)
                bb = nop.tile([128, NBLK], F32, name="ln_b")[:, 0:nb]
                nc.vector.tensor_tensor(out=bb, in0=s1, in1=r,
                                        op=MUL)
                # u = h*A - B ; v = u*g + b ; out = relu(v)
                nc.vector.tensor_tensor(out=u, in0=src_tile,
                                        in1=a.broadcast_to([128, nb, HID]),
                                        op=MUL)
                nc.vector.tensor_tensor(out=u, in0=u,
                                        in1=bb.broadcast_to([128, nb, HID]),
                                        op=SUB)
                nc.vector.tensor_tensor(out=u, in0=u,
                                        in1=_bcast_mid(g_ap, nb), op=MUL)
                nc.vector.tensor_tensor(out=u, in0=u,
                                        in1=_bcast_mid(b_ap, nb), op=ADD)
                nc.scalar.activation(out=u, in_=u,
                                     func=mybir.ActivationFunctionType.Relu)

            def emit_ln_ag(li, half):
                """x = relu(LN(h)) for one half of the blocks, then DMA to
                xin and AllGather into that half's shard buffer."""
                b0 = half * HBLK
                nb = HBLK if half == 0 else NBLK - HBLK
                batched_ln_relu(bias_sb["ng"][:, li, :],
                                bias_sb["nb"][:, li, :], b0, nb)
                nc.vector.tensor_copy(out=xbf[:, b0:b0 + nb, :],
                                      in_=_sn_half(HID, b0, nb))
                xin = xins[li][half]
                nc.sync.dma_start(
                    out=xin[:, :].rearrange("(b p) k -> p b k", p=128),
                    in_=xbf[:, b0:b0 + nb, :])
                nc.gpsimd.collective_compute(
                    "AllGather", mybir.AluOpType.bypass,
                    replica_groups=[list(range(N_CORES))],
                    ins=[xin[:, :].opt()], outs=[xfulls[li][half][:, :].opt()])

            # half-graph boundary: first group index covering block HBLK-1
            g_half_idx = next(gi for gi, grp in enumerate(groups)
                              if grp["b0"] + grp["nb"] >= HBLK)

            # ================= layers =================
            emit_ln_ag(0, 0)
            emit_ln_ag(0, 1)
            for li in range(n_layers):
                beta = float(betas[li])
                xfA, xfB = xfulls[li]

                # ---- MLP per block: h += W2 @ relu(LN1(y@W1+b1)) + b2 ----
                def mlp_block(b):
                    pyt = pmlp.tile([HID, 128], F32, name="pyt", tag="ptr",
                                    bufs=2)
                    nc.tensor.transpose(out=pyt[:], in_=sn[:, b, HID:2 * HID],
                                        identity=ident[:])
                    yt = nop.tile([HID, 128], F32, name="yt")
                    nc.scalar.activation(out=yt[:], in_=pyt[:],
                                         func=mybir.ActivationFunctionType.Copy)
                    pm1 = pmlp.tile([128, 2 * HID], F32, name="pm1", tag="pmm",
                                    bufs=2)
                    nc.tensor.matmul(out=pm1[:], lhsT=yt[:],
                                     rhs=w1_sb[:, li, :], start=True, stop=True)
                    z0 = nop.tile([128, 2 * HID], F32, name="z0")
                    nc.vector.tensor_tensor(out=z0[:], in0=pm1[:],
                                            in1=bias_sb["b1"][:, li, :],
                                            op=ADD)
                    st1 = nop.tile([128, 6], F32, name="st1")
                    nc.vector.bn_stats(out=st1[:], in_=z0[:])
                    mv1 = nop.tile([128, 2], F32, name="mv1")
                    nc.vector.bn_aggr(out=mv1[:], in_=st1[:])
                    sd = nop.tile([128, 1], F32, name="sd")
                    nc.scalar.activation(out=sd[:], in_=mv1[:, 1:2],
                                         func=mybir.ActivationFunctionType.Sqrt,
                                         bias=eps_sb[:])
                    rstd = nop.tile([128, 1], F32, name="rstd")
                    nc.vector.reciprocal(out=rstd[:], in_=sd[:])
                    z1 = nop.tile([128, 2 * HID], F32, name="z1")
                    nc.vector.scalar_tensor_tensor(
                        out=z1[:], in0=z0[:], scalar=mv1[:, 0:1],
                        in1=rstd[:].broadcast_to([128, 2 * HID]),
                        op0=SUB, op1=MUL)
                    nc.vector.tensor_tensor(out=z1[:], in0=z1[:],
                                            in1=bias_sb["l1g"][:, li, :],
                                            op=MUL)
                    nc.vector.tensor_tensor(out=z1[:], in0=z1[:],
                                            in1=bias_sb["l1b"][:, li, :],
                                            op=ADD)
                    nc.scalar.activation(out=z1[:], in_=z1[:],
                                         func=mybir.ActivationFunctionType.Relu)
                    pzt = pmlp.tile([128, 128], F32, name="pzt", tag="ptr",
                                    bufs=2)
                    nc.tensor.transpose(out=pzt[:], in_=z1[:], identity=ident[:])
                    zt = nop.tile([128, 128], F32, name="zt")
                    nc.scalar.activation(out=zt[:], in_=pzt[:],
                                         func=mybir.ActivationFunctionType.Copy)
                    pm2 = pmlp.tile([128, HID], F32, name="pm2", tag="pmm",
                                    bufs=2)
                    nc.tensor.matmul(out=pm2[:], lhsT=zt[:],
                                     rhs=w2_sb[:, li, :], start=True, stop=True)
                    y2 = nop.tile([128, HID], F32, name="y2")
                    nc.vector.tensor_tensor(out=y2[:], in0=pm2[:],
                                            in1=bias_sb["b2"][:, li, :],
                                            op=ADD)
                    nc.vector.tensor_tensor(out=h_sb[:, b, :],
                                            in0=h_sb[:, b, :], in1=y2[:],
                                            op=ADD)

                # ---- edge phase: per group ----
                for gi, grp in enumerate(groups):
                    c0 = grp["c0"]
                    jg = grp["js0"] + grp["js1"]
                    b0, nb = grp["b0"], grp["nb"]
                    est = wk.tile([128, jgmax, EW], BF16, name="est", bufs=3)
                    eng = nc.sync if gi % 2 == 0 else nc.scalar
                    eng.dma_start(
                        out=est[:, 0:jg, :].rearrange("p j k -> p (j k)"),
                        in_=est_d[:, c0 * EW:(c0 + jg) * EW])
                    idxg = wk.tile([128, jgmax * 8], I16, name="idxg",
                                   bufs=3)
                    nc.sync.dma_start(out=idxg[:, 0:jg * 8],
                                      in_=idx_d[:, c0 * 8:(c0 + jg) * 8])
                    xq = wk.tile([128, jgmax, 2 * HID], BF16, name="xq",
                                 bufs=3)
                    # gathers: per shard section, in <=PIECE-chunk pieces,
                    # round-robin across both SWDGE queues (parallel Q7 gen)
                    for s, (soff, scnt) in enumerate(
                            [(0, grp["js0"]), (grp["js0"], grp["js1"])]):
                        p0 = 0
                        while p0 < scnt:
                            n = min(PIECE, scnt - p0)
                            o = soff + p0
                            nc.gpsimd.dma_gather(
                                out_ap=xq[:, o:o + n, :],
                                in_ap=(xfA if s == 0 else xfB)[:, :],
                                idxs_ap=idxg[:, o * 8:(o + n) * 8],
                                num_idxs=n * 128,
                                num_idxs_reg=nidx_reg(n * 128),
                                elem_size=2 * HID,
                                queue_num=qrr[0] % 2,
                            )
                            qrr[0] += 1
                            p0 += n
                    # parity-select x half: xsel = lo + par*(hi-lo)
                    xq_lo = xq[:, 0:jg, 0:HID]
                    xq_hi = xq[:, 0:jg, HID:2 * HID]
                    nc.vector.tensor_tensor(out=xq_hi, in0=xq_hi, in1=xq_lo,
                                            op=SUB)
                    parb = AP(par_sb[:].tensor, par_sb[:].offset + c0,
                              [list(par_sb[:].ap[0]), [1, jg], [0, HID]])
                    nc.vector.tensor_tensor(out=xq_hi, in0=xq_hi, in1=parb,
                                            op=MUL)
                    nc.vector.tensor_tensor(out=xq_hi, in0=xq_hi, in1=xq_lo,
                                            op=ADD)
                    # z = x[src] + eh (into est's eh half)
                    nc.vector.tensor_tensor(out=est[:, 0:jg, 0:HID],
                                            in0=xq_hi,
                                            in1=est[:, 0:jg, 0:HID], op=ADD)
                    # m = relu(z) in place
                    nc.vector.tensor_scalar_max(out=est[:, 0:jg, 0:HID],
                                                in0=est[:, 0:jg, 0:HID],
                                                scalar1=0.0)
                    em = wk.tile([128, jgmax, 2 * HID], BF16, name="em",
                                 bufs=3)
                    nc.scalar.activation(out=em[:, 0:jg, 0:HID],
                                         in_=est[:, 0:jg, 0:HID],
                                         func=mybir.ActivationFunctionType.Exp,
                                         scale=beta)
                    nc.vector.tensor_tensor(out=em[:, 0:jg, HID:2 * HID],
                                            in0=est[:, 0:jg, 0:HID],
                                            in1=em[:, 0:jg, 0:HID], op=MUL)
                    # segment sums on PE: one-hot matmuls, accumulate per block
                    for bi, (b, pos) in enumerate(grp["blocks"]):
                        psb = pseg.tile([128, 2 * HID], F32, name="psb")
                        for ci, cpos in enumerate(pos):
                            nc.tensor.matmul(out=psb[:],
                                             lhsT=est[:, cpos, HID:EW],
                                             rhs=em[:, cpos, :],
                                             start=(ci == 0),
                                             stop=(ci == len(pos) - 1))
                        nc.vector.tensor_copy(out=sn[:, b, :], in_=psb[:])
                    # epilogue for this group: y = num/max(den,.5) + eps + x
                    den = _sn_half(0, b0, nb)
                    num = _sn_half(HID, b0, nb)
                    nc.vector.tensor_scalar_max(out=den, in0=den, scalar1=0.5)
                    nc.vector.reciprocal(out=den, in_=den)
                    nc.vector.tensor_tensor(out=num, in0=num, in1=den, op=MUL)
                    nc.vector.tensor_scalar_add(out=num, in0=num,
                                                scalar1=EPS_MSG)
                    nc.vector.tensor_tensor(out=num, in0=num,
                                            in1=xbf[:, b0:b0 + nb, :], op=ADD)
                    for b, _pos in grp["blocks"]:
                        mlp_block(b)
                    if li + 1 < n_layers and gi == g_half_idx:
                        emit_ln_ag(li + 1, 0)
                if li + 1 < n_layers:
                    emit_ln_ag(li + 1, 1)

            # ================= final head =================
            batched_ln_relu(bias_sb["ng"][:, 0, :], bias_sb["nb"][:, 0, :])
            OB = 7
            for ob0 in range(0, NBLK, OB):
                onb = min(OB, NBLK - ob0)
                oc = wk.tile([128, OB, OUT_DIM], F32, name="oc")
                for i in range(onb):
                    b = ob0 + i
                    pxt = pmlp.tile([HID, 128], F32, name="pxt", tag="ptr",
                                    bufs=2)
                    nc.tensor.transpose(out=pxt[:], in_=sn[:, b, HID:2 * HID],
                                        identity=ident[:])
                    xt = nop.tile([HID, 128], F32, name="xt")
                    nc.scalar.activation(out=xt[:], in_=pxt[:],
                                         func=mybir.ActivationFunctionType.Copy)
                    po = pmlp.tile([128, OUT_DIM], F32, name="po", tag="pmm",
                                   bufs=2)
                    nc.tensor.matmul(out=po[:], lhsT=xt[:], rhs=wout_sb[:],
                                     start=True, stop=True)
                    nc.vector.tensor_tensor(out=oc[:, i, :], in0=po[:],
                                            in1=bias_sb["bout"][:],
                                            op=ADD)
                nc.sync.dma_start(
                    out=out_d[ob0 * 128:(ob0 + onb) * 128, :].rearrange(
                        "(b p) k -> p b k", p=128),
                    in_=oc[:, 0:onb, :])

    lower_extended_insts(nc)
    _install_wait_split(nc)
    wb_np = {k[1:-3]: v for k, v in wb_d.items() if k.startswith("_")}
    return nc, wb_np


def prepare(inputs, n_layers=L_LAYERS):
    """Returns (nc, in_maps) ready for run_bass_kernel_spmd."""
    node_feats = np.asarray(inputs["node_feats"], np.float32)
    edge_feats = np.asarray(inputs["edge_feats"], np.float32)
    src = np.asarray(inputs["src"], np.int32)
    dst = np.asarray(inputs["dst"], np.int32)
    cores, groups, ncht = host_prep(
        node_feats, edge_feats, src, dst,
        np.asarray(inputs["W_node"], np.float32),
        np.asarray(inputs["b_node"], np.float32),
        np.asarray(inputs["W_edge"], np.float32),
        np.asarray(inputs["b_edge"], np.float32))
    weights = tuple(np.asarray(inputs[k], np.float32) for k in
                    ["betas", "W1", "b1", "ln1_g", "ln1_b", "W2", "b2",
                     "norm_g", "norm_b", "W_out", "b_out"])
    nc, wb_np = build_bass(weights, groups, ncht, n_layers=n_layers)
    in_maps = []
    for k in range(N_CORES):
        c = cores[k]
        m = dict(h0=np.ascontiguousarray(c["h0"].reshape(128, NBLK * HID)),
                 est=np.ascontiguousarray(c["est"].reshape(128, ncht * EW)),
                 idx16=c["idx16"], par=c["par"])
        m.update(wb_np)
        in_maps.append(m)
    return nc, in_maps


def kernel(node_feats, edge_feats, src, dst, W_node, b_node, W_edge, b_edge,
           betas, W1, b1, ln1_g, ln1_b, W2, b2, norm_g, norm_b, W_out, b_out,
           n_layers=L_LAYERS):
    inputs = dict(node_feats=node_feats, edge_feats=edge_feats, src=src,
                  dst=dst, W_node=W_node, b_node=b_node, W_edge=W_edge,
                  b_edge=b_edge, betas=betas, W1=W1, b1=b1, ln1_g=ln1_g,
                  ln1_b=ln1_b, W2=W2, b2=b2, norm_g=norm_g, norm_b=norm_b,
                  W_out=W_out, b_out=b_out)
    nc, in_maps = prepare(inputs, n_layers=n_layers)
    res = run_bass_kernel_spmd(nc, in_maps, core_ids=list(range(N_CORES)))

    out = np.zeros((N_NODES, OUT_DIM), np.float32)
    for k in range(N_CORES):
        o = res.results[k]["out"]
        out[k * N_PER_CORE:(k + 1) * N_PER_CORE] = o[:N_PER_CORE]
    return out
